# revision 10
# baseline (speedup 1.0000x reference)
"""Two-launch Trainium2 kernel for nn_DualStreamPhasorBlock.

Sharding: 8 cores = (batch b in {0,1}) x (sequence chunk c in {0..3}, 512 rows).
Launch 1: per-core local work (projections, phasor trig, intra-chunk linear
attention, local pos cumsum, gates) + per-chunk summary states.
Host: tiny exclusive prefix-sum of the (64+2, 256) states across chunks.
Launch 2: apply cross-chunk carries, LayerNorm, output projection, residual.
"""
import sys, math, types
sys.path.insert(0, "/opt/trn_rl_repo")
import numpy as np
import ml_dtypes

from concourse import bacc, tile, mybir
from concourse.bass_utils import run_bass_kernel_spmd

F32 = mybir.dt.float32
BF16 = mybir.dt.bfloat16
BF = ml_dtypes.bfloat16
PI = math.pi
D, K, B, L = 256, 32, 2, 2048
CH, NB = 512, 4           # rows per core; 128-row blocks per core
CC = 1.5 * 2 ** 23        # fp32 round-to-int magic constant
AOP = mybir.AluOpType
AFT = mybir.ActivationFunctionType

PROFILE = {"trace": False, "exec_ns": []}


def _install_shim():
    """Register NTFF profile hook (image's antenv lacks axon_hooks)."""
    try:
        import antenv
        if "antenv.axon_hooks" not in sys.modules:
            from trn_agent_boot import trn_boot
            hook = trn_boot._ntff_profile_via_ctypes("/opt/axon/libaxon_pjrt.so")
            mod = types.ModuleType("antenv.axon_hooks")
            mod.get_axon_ntff_profile_hook = lambda: hook
            mod.set_axon_ntff_profile_hook = lambda h: None
            sys.modules["antenv.axon_hooks"] = mod
            antenv.axon_hooks = mod
        from concourse import bass_utils
        bass_utils.upload_artifacts = lambda tmpdir: f"local:{tmpdir}"
    except Exception:
        pass


def _build_l1():
    nc = bacc.Bacc("TRN2", target_bir_lowering=False, debug=False, num_devices=8)
    dp = nc.declare_dram_parameter
    xT_e = dp("xT", [D, CH], BF16, isOutput=False)
    ph_e = dp("ph", [CH, D], F32, isOutput=False)
    wk1_e = dp("wk1", [D, D], BF16, isOutput=False)
    wq1_e = dp("wq1", [D, D], BF16, isOutput=False)
    wk2_e = dp("wk2", [D, K], BF16, isOutput=False)
    wq2_e = dp("wq2", [D, K], BF16, isOutput=False)
    wvc_e = dp("wvc", [D, D], BF16, isOutput=False)
    wvp_e = dp("wvp", [D, D], BF16, isOutput=False)
    wg1_e = dp("wg1", [D, 64], BF16, isOutput=False)
    wg2_e = dp("wg2", [64, 2], BF16, isOutput=False)
    bk1_e = dp("bk1", [128, 2], F32, isOutput=False)   # col per h-tile
    bq1_e = dp("bq1", [128, 2], F32, isOutput=False)
    bkq2_e = dp("bkq2", [64, 1], F32, isOutput=False)  # [bk2; bq2]
    bg1_e = dp("bg1", [64, 1], F32, isOutput=False)
    bvc_e = dp("bvcr", [1, D], BF16, isOutput=False)
    bvp_e = dp("bvpr", [1, D], BF16, isOutput=False)
    bg2_e = dp("bg2r", [1, 2], BF16, isOutput=False)
    tri_e = dp("trif", [128, 128], F32, isOutput=False)    # t<=l upper(incl diag)
    trib_e = dp("trib", [128, 128], BF16, isOutput=False)
    onesr_e = dp("onesr", [1, 128], BF16, isOutput=False)
    onesc_e = dp("onesc", [128, 1], BF16, isOutput=False)
    idn64_e = dp("idn64", [64, 64], BF16, isOutput=False)
    isqp_e = dp("isqp", [128, NB], F32, isOutput=False)    # 1/sqrt(pos)
    isqpk_e = dp("isqpk", [128, NB], F32, isOutput=False)  # 1/sqrt(pos*K)

    comb_o = dp("comb", [CH, D], F32, isOutput=True)
    qf_o = dp("qfo", [64, CH], BF16, isOutput=True)
    cosp_o = dp("cospo", [CH, D], BF16, isOutput=True)
    sinp_o = dp("sinpo", [CH, D], BF16, isOutput=True)
    g0_o = dp("g0o", [128, NB], F32, isOutput=True)
    g1_o = dp("g1o", [128, NB], F32, isOutput=True)
    st_o = dp("sto", [66, D], F32, isOutput=True)

    with tile.TileContext(nc) as tc:
        with (
            tc.tile_pool(name="cst", bufs=1) as cst,
            tc.tile_pool(name="sb", bufs=1) as sb,
            tc.tile_pool(name="sc", bufs=2) as sc,
        ):
            # ---- constants / loads ----
            def load(pool, shape, dt, src, tag=None):
                t = pool.tile(shape, dt, tag=tag or ("ld_" + src.name))
                nc.sync.dma_start(t[:], src[:])
                return t

            xT = [load(sb, [128, CH], BF16, xT_e[i * 128:(i + 1) * 128, :], tag=f"xT{i}")
                  for i in range(2)]
            ph = [load(sb, [128, D], F32, ph_e[i * 128:(i + 1) * 128, :], tag=f"ph{i}")
                  for i in range(NB)]
            wk1 = [load(sb, [128, D], BF16, wk1_e[i * 128:(i + 1) * 128, :], tag=f"wk1{i}") for i in range(2)]
            wq1 = [load(sb, [128, D], BF16, wq1_e[i * 128:(i + 1) * 128, :], tag=f"wq1{i}") for i in range(2)]
            wk2 = [load(sb, [128, K], BF16, wk2_e[i * 128:(i + 1) * 128, :], tag=f"wk2{i}") for i in range(2)]
            wq2 = [load(sb, [128, K], BF16, wq2_e[i * 128:(i + 1) * 128, :], tag=f"wq2{i}") for i in range(2)]
            wvc = [load(sb, [128, D], BF16, wvc_e[i * 128:(i + 1) * 128, :], tag=f"wvc{i}") for i in range(2)]
            wvp = [load(sb, [128, D], BF16, wvp_e[i * 128:(i + 1) * 128, :], tag=f"wvp{i}") for i in range(2)]
            wg1 = [load(sb, [128, 64], BF16, wg1_e[i * 128:(i + 1) * 128, :], tag=f"wg1{i}") for i in range(2)]
            wg2 = load(sb, [64, 2], BF16, wg2_e)
            bk1 = load(cst, [128, 2], F32, bk1_e)
            bq1 = load(cst, [128, 2], F32, bq1_e)
            bkq2 = load(cst, [64, 1], F32, bkq2_e)
            bg1 = load(cst, [64, 1], F32, bg1_e)
            bvcr = load(cst, [1, D], BF16, bvc_e)
            bvpr = load(cst, [1, D], BF16, bvp_e)
            bg2r = load(cst, [1, 2], BF16, bg2_e)
            trif = load(cst, [128, 128], F32, tri_e)
            trib = load(cst, [128, 128], BF16, trib_e)
            onesr = load(cst, [1, 128], BF16, onesr_e)
            onesc = load(cst, [128, 1], BF16, onesc_e)
            idn64 = load(cst, [64, 64], BF16, idn64_e)
            isqp = load(cst, [128, NB], F32, isqp_e)
            isqpk = load(cst, [128, NB], F32, isqpk_e)

            c_pi2 = cst.tile([128, 1], F32, tag="c_pi2")
            nc.gpsimd.memset(c_pi2[:], PI / 2)
            c_cc = cst.tile([128, 1], F32, tag="c_cc")
            nc.gpsimd.memset(c_cc[:], CC)

            # ---- B: hidden layers (feature-major) ----
            psb_ctx = tc.tile_pool(name="psb", bufs=2, space="PSUM")
            psb = psb_ctx.__enter__()
            hk, hq = [], []
            for mt in range(2):
                p = psb.tile([128, CH], F32, tag="big")
                nc.tensor.matmul(p[:], wk1[0][:, mt * 128:(mt + 1) * 128], xT[0][:], start=True, stop=False)
                nc.tensor.matmul(p[:], wk1[1][:, mt * 128:(mt + 1) * 128], xT[1][:], start=False, stop=True)
                h = sb.tile([128, CH], BF16, tag=f"hk{mt}")
                nc.scalar.activation(h[:], p[:], AFT.Tanh, bias=bk1[:, mt:mt + 1])
                hk.append(h)
            for mt in range(2):
                p = psb.tile([128, CH], F32, tag="big")
                nc.tensor.matmul(p[:], wq1[0][:, mt * 128:(mt + 1) * 128], xT[0][:], start=True, stop=False)
                nc.tensor.matmul(p[:], wq1[1][:, mt * 128:(mt + 1) * 128], xT[1][:], start=False, stop=True)
                h = sb.tile([128, CH], BF16, tag=f"hq{mt}")
                nc.scalar.activation(h[:], p[:], AFT.Tanh, bias=bq1[:, mt:mt + 1])
                hq.append(h)

            # ---- C: phase layers + trig -> KF, QF (64,512) bf16 ----
            kq = psb.tile([64, CH], F32, tag="big")
            nc.tensor.matmul(kq[0:32, :], wk2[0], hk[0][:], start=True, stop=False)
            nc.tensor.matmul(kq[0:32, :], wk2[1], hk[1][:], start=False, stop=True)
            nc.tensor.matmul(kq[32:64, :], wq2[0], hq[0][:], start=True, stop=False)
            nc.tensor.matmul(kq[32:64, :], wq2[1], hq[1][:], start=False, stop=True)
            tkq = sb.tile([64, CH], F32, tag="tkq")
            nc.scalar.activation(tkq[:], kq[:], AFT.Tanh, bias=bkq2[:])
            s2c = sb.tile([64, CH], F32, tag="s2c")
            nc.scalar.activation(s2c[:], tkq[:], AFT.Sin, scale=PI / 2)
            c2c = sb.tile([64, CH], F32, tag="c2c")
            nc.scalar.activation(c2c[:], tkq[:], AFT.Sin, bias=c_pi2[0:64, :], scale=PI / 2)
            q2c = sb.tile([64, CH], F32, tag="q2c")
            nc.scalar.activation(q2c[:], s2c[:], AFT.Square)
            KF = sb.tile([64, CH], BF16, tag="KF")
            QF = sb.tile([64, CH], BF16, tag="QF")
            # sin = 2*s2*c2 ; cos = 1 - 2*s2^2
            nc.vector.scalar_tensor_tensor(KF[32:64, :], s2c[0:32, :], 2.0, c2c[0:32, :], AOP.mult, AOP.mult)
            nc.vector.scalar_tensor_tensor(QF[32:64, :], s2c[32:64, :], 2.0, c2c[32:64, :], AOP.mult, AOP.mult)
            nc.vector.tensor_scalar(KF[0:32, :], q2c[0:32, :], -2.0, 1.0, AOP.mult, AOP.add)
            nc.vector.tensor_scalar(QF[0:32, :], q2c[32:64, :], -2.0, 1.0, AOP.mult, AOP.add)
            nc.sync.dma_start(qf_o[:], QF[:])

            # ---- H: gates (hidden layer, then close psb pool) ----
            hgp = psb.tile([64, CH], F32, tag="big")
            nc.tensor.matmul(hgp[:], wg1[0], xT[0][:], start=True, stop=False)
            nc.tensor.matmul(hgp[:], wg1[1], xT[1][:], start=False, stop=True)
            hg = sb.tile([64, CH], BF16, tag="hg")
            nc.scalar.activation(hg[:], hgp[:], AFT.Relu, bias=bg1[:])
            psb_ctx.__exit__(None, None, None)
            psm_ctx = tc.tile_pool(name="psm", bufs=4, space="PSUM")
            psm = psm_ctx.__enter__()
            g0p = sb.tile([128, NB], F32, tag="g0p")
            g1p = sb.tile([128, NB], F32, tag="g1p")
            for j in range(NB):
                sl = slice(j * 128, (j + 1) * 128)
                lgp = psm.tile([128, 2], F32, tag="med")
                nc.tensor.matmul(lgp[:], hg[:, sl], wg2[:], start=True, stop=False)
                nc.tensor.matmul(lgp[:], onesr[:], bg2r[:], start=False, stop=True)
                lg = sc.tile([128, 2], F32, tag="lg")
                nc.vector.tensor_copy(lg[:], lgp[:])
                df = sc.tile([128, 1], F32, tag="df")
                nc.vector.tensor_sub(df[:], lg[:, 0:1], lg[:, 1:2])
                g0 = sc.tile([128, 1], F32, tag="g0")
                nc.scalar.activation(g0[:], df[:], AFT.Sigmoid)
                g1 = sc.tile([128, 1], F32, tag="g1")
                nc.scalar.activation(g1[:], df[:], AFT.Sigmoid, scale=-1.0)
                nc.vector.tensor_mul(g0p[:, j:j + 1], g0[:], isqp[:, j:j + 1])
                nc.vector.tensor_mul(g1p[:, j:j + 1], g1[:], isqpk[:, j:j + 1])
            nc.sync.dma_start(g0_o[:], g0p[:])
            nc.sync.dma_start(g1_o[:], g1p[:])

            # ---- D: values ----
            v, vp = [], []
            for j in range(NB):
                sl = slice(j * 128, (j + 1) * 128)
                pv = psm.tile([128, D], F32, tag="med")
                nc.tensor.matmul(pv[:], xT[0][:, sl], wvc[0][:], start=True, stop=False)
                nc.tensor.matmul(pv[:], xT[1][:, sl], wvc[1][:], start=False, stop=False)
                nc.tensor.matmul(pv[:], onesr[:], bvcr[:], start=False, stop=True)
                vt = sb.tile([128, D], BF16, tag=f"v{j}")
                nc.scalar.copy(vt[:], pv[:])
                v.append(vt)
                pw = psm.tile([128, D], F32, tag="med")
                nc.tensor.matmul(pw[:], xT[0][:, sl], wvp[0][:], start=True, stop=False)
                nc.tensor.matmul(pw[:], xT[1][:, sl], wvp[1][:], start=False, stop=False)
                nc.tensor.matmul(pw[:], onesr[:], bvpr[:], start=False, stop=True)
                wt = sb.tile([128, D], BF16, tag=f"vp{j}")
                nc.scalar.copy(wt[:], pw[:])
                vp.append(wt)

            # ---- E: KF row-major + state chain ----
            Ssb = []          # Ssb[j] = f32 local state after blocks 0..j
            Ssbb = []         # bf16 casts: Ssbb[j-1] = state before block j
            for j in range(NB):
                sl = slice(j * 128, (j + 1) * 128)
                tp = psm.tile([128, 64], BF16, tag="medt", bufs=2)
                nc.tensor.transpose(tp[:], KF[:, sl], idn64[:])
                kfr = sc.tile([128, 64], BF16, tag="kfr")
                nc.vector.tensor_copy(kfr[:], tp[:])
                sp = psm.tile([64, D], F32, tag="med")
                nc.tensor.matmul(sp[:], kfr[:], v[j][:], start=True, stop=True)
                s1 = sb.tile([64, D], F32, tag=f"Ssb{j}")
                if j == 0:
                    nc.vector.tensor_copy(s1[:], sp[:])
                else:
                    nc.vector.tensor_add(s1[:], Ssb[-1][:], sp[:])
                Ssb.append(s1)
                if j < NB - 1:
                    sbf = sb.tile([64, D], BF16, tag=f"Sbf{j}")
                    nc.vector.tensor_copy(sbf[:], s1[:])
                    Ssbb.append(sbf)
            nc.sync.dma_start(st_o[0:64, :], Ssb[3][:])  # total local state

            # ---- F+G interleaved per block ----
            lcr = None
            lci = None
            for j in range(NB):
                sl = slice(j * 128, (j + 1) * 128)
                # scores + mask
                ap = psm.tile([128, 128], F32, tag="med")
                nc.tensor.matmul(ap[:], KF[:, sl], QF[:, sl], start=True, stop=True)
                am = sc.tile([128, 128], BF16, tag="am")
                nc.vector.tensor_mul(am[:], ap[:], trif[:])
                # content psum: intra + local inter
                op = psm.tile([128, D], F32, tag="med")
                nc.tensor.matmul(op[:], am[:], v[j][:], start=True, stop=(j == 0))
                if j > 0:
                    nc.tensor.matmul(op[:], QF[:, sl], Ssbb[j - 1][:], start=False, stop=True)
                # pos trig
                y = sc.tile([128, D], F32, tag="y")
                nc.scalar.activation(y[:], ph[j][:], AFT.Identity, bias=c_cc[:], scale=1.0 / (2 * PI))
                t = sc.tile([128, D], F32, tag="t")
                nc.vector.tensor_scalar(t[:], y[:], CC, -2 * PI, AOP.subtract, AOP.mult)
                yr = sc.tile([128, D], F32, tag="yr")
                nc.vector.tensor_add(yr[:], ph[j][:], t[:])
                s2 = sc.tile([128, D], F32, tag="s2")
                nc.scalar.activation(s2[:], yr[:], AFT.Sin, scale=0.5)
                c2 = sc.tile([128, D], F32, tag="c2")
                nc.scalar.activation(c2[:], yr[:], AFT.Sin, bias=c_pi2[:], scale=0.5)
                q2 = sc.tile([128, D], F32, tag="q2")
                nc.scalar.activation(q2[:], s2[:], AFT.Square)
                sinp = sb.tile([128, D], BF16, tag=f"sinp{j}")
                nc.vector.scalar_tensor_tensor(sinp[:], s2[:], 2.0, c2[:], AOP.mult, AOP.mult)
                cosp = sb.tile([128, D], BF16, tag=f"cosp{j}")
                nc.vector.tensor_scalar(cosp[:], q2[:], -2.0, 1.0, AOP.mult, AOP.add)
                nc.sync.dma_start(sinp_o[sl, :], sinp[:])
                nc.sync.dma_start(cosp_o[sl, :], cosp[:])
                ur = sc.tile([128, D], BF16, tag="ur")
                nc.vector.tensor_mul(ur[:], vp[j][:], cosp[:])
                ui = sc.tile([128, D], BF16, tag="ui")
                nc.vector.tensor_mul(ui[:], vp[j][:], sinp[:])
                # pos mem (tri cumsum + carry)
                mr = psm.tile([128, D], F32, tag="med")
                nc.tensor.matmul(mr[:], trib[:], ur[:], start=True, stop=(j == 0))
                if j > 0:
                    nc.tensor.matmul(mr[:], onesr[:], lcr[:], start=False, stop=True)
                mi = psm.tile([128, D], F32, tag="med")
                nc.tensor.matmul(mi[:], trib[:], ui[:], start=True, stop=(j == 0))
                if j > 0:
                    nc.tensor.matmul(mi[:], onesr[:], lci[:], start=False, stop=True)
                # carry chain via column sums (ones-col matmul)
                csr = psm.tile([1, D], F32, tag="row", bufs=2)
                nc.tensor.matmul(csr[:], onesc[:], ur[:], start=True, stop=True)
                csi = psm.tile([1, D], F32, tag="row", bufs=2)
                nc.tensor.matmul(csi[:], onesc[:], ui[:], start=True, stop=True)
                if j < NB - 1:
                    ncr = sb.tile([1, D], BF16, tag=f"lcr{j}")
                    nci = sb.tile([1, D], BF16, tag=f"lci{j}")
                    if j == 0:
                        nc.vector.tensor_copy(ncr[:], csr[:])
                        nc.vector.tensor_copy(nci[:], csi[:])
                    else:
                        nc.vector.tensor_add(ncr[:], lcr[:], csr[:])
                        nc.vector.tensor_add(nci[:], lci[:], csi[:])
                    lcr, lci = ncr, nci
                else:
                    fr = sb.tile([1, D], F32, tag="totr")
                    nc.vector.tensor_add(fr[:], lcr[:], csr[:])
                    fi = sb.tile([1, D], F32, tag="toti")
                    nc.vector.tensor_add(fi[:], lci[:], csi[:])
                    nc.sync.dma_start(st_o[64:65, :], fr[:])
                    nc.sync.dma_start(st_o[65:66, :], fi[:])
                # combine: comb = g0p*(mr*cosp + mi*sinp) + g1p*content
                t1 = sc.tile([128, D], F32, tag="t1")
                nc.vector.scalar_tensor_tensor(t1[:], mr[:], g0p[:, j:j + 1], cosp[:], AOP.mult, AOP.mult)
                t2 = sc.tile([128, D], F32, tag="t2")
                nc.vector.scalar_tensor_tensor(t2[:], mi[:], g0p[:, j:j + 1], sinp[:], AOP.mult, AOP.mult)
                a = sc.tile([128, D], F32, tag="a")
                nc.vector.scalar_tensor_tensor(a[:], op[:], g1p[:, j:j + 1], t1[:], AOP.mult, AOP.add)
                comb = sc.tile([128, D], F32, tag="comb")
                nc.vector.tensor_add(comb[:], a[:], t2[:])
                nc.sync.dma_start(comb_o[sl, :], comb[:])
            psm_ctx.__exit__(None, None, None)
    nc.compile()
    return nc


def _build_l2():
    nc = bacc.Bacc("TRN2", target_bir_lowering=False, debug=False, num_devices=8)
    dp = nc.declare_dram_parameter
    comb_e = dp("comb", [CH, D], F32, isOutput=False)
    qf_e = dp("qf", [64, CH], BF16, isOutput=False)
    cosp_e = dp("cosp", [CH, D], BF16, isOutput=False)
    sinp_e = dp("sinp", [CH, D], BF16, isOutput=False)
    scar_e = dp("scar", [64, D], BF16, isOutput=False)
    pcar_e = dp("pcar", [2, D], BF16, isOutput=False)
    g0_e = dp("g0", [128, NB], F32, isOutput=False)
    g1_e = dp("g1", [128, NB], F32, isOutput=False)
    x_e = dp("x", [CH, D], F32, isOutput=False)
    wo_e = dp("wo", [D, D], BF16, isOutput=False)
    bo_e = dp("bor", [1, D], BF16, isOutput=False)
    onesr_e = dp("onesr", [1, 128], BF16, isOutput=False)
    idn_e = dp("idn", [128, 128], BF16, isOutput=False)
    out_o = dp("out", [CH, D], F32, isOutput=True)

    with tile.TileContext(nc) as tc:
        with (
            tc.tile_pool(name="cst", bufs=1) as cst,
            tc.tile_pool(name="sb", bufs=1) as sb,
            tc.tile_pool(name="sc", bufs=2) as sc,
            tc.tile_pool(name="psm", bufs=4, space="PSUM") as psm,
        ):
            def load(pool, shape, dt, src, tag=None):
                t = pool.tile(shape, dt, tag=tag or ("ld_" + src.name))
                nc.sync.dma_start(t[:], src[:])
                return t

            qf = load(sb, [64, CH], BF16, qf_e)
            scar = load(sb, [64, D], BF16, scar_e)
            pcar_r = load(sb, [1, D], BF16, pcar_e[0:1, :], tag="pcar_r")
            pcar_i = load(sb, [1, D], BF16, pcar_e[1:2, :], tag="pcar_i")
            g0p = load(sb, [128, NB], F32, g0_e)
            g1p = load(sb, [128, NB], F32, g1_e)
            wo = [load(sb, [128, D], BF16, wo_e[i * 128:(i + 1) * 128, :], tag=f"wo{i}") for i in range(2)]
            bor = load(cst, [1, D], BF16, bo_e)
            onesr = load(cst, [1, 128], BF16, onesr_e)
            idn = load(cst, [128, 128], BF16, idn_e)
            combs = [load(sb, [128, D], F32, comb_e[j * 128:(j + 1) * 128, :], tag=f"cb{j}") for j in range(NB)]
            cosps = [load(sb, [128, D], BF16, cosp_e[j * 128:(j + 1) * 128, :], tag=f"cp{j}") for j in range(NB)]
            sinps = [load(sb, [128, D], BF16, sinp_e[j * 128:(j + 1) * 128, :], tag=f"sp{j}") for j in range(NB)]
            xs = [load(sb, [128, D], F32, x_e[j * 128:(j + 1) * 128, :], tag=f"x{j}") for j in range(NB)]
            c_eps = cst.tile([128, 1], F32, tag="c_eps")
            nc.gpsimd.memset(c_eps[:], 1e-5)

            # broadcast pos carries to (128, D)
            pbr_p = psm.tile([128, D], F32, tag="med")
            nc.tensor.matmul(pbr_p[:], onesr[:], pcar_r[:], start=True, stop=True)
            pbr = sb.tile([128, D], BF16, tag="pbr")
            nc.scalar.copy(pbr[:], pbr_p[:])
            pbi_p = psm.tile([128, D], F32, tag="med")
            nc.tensor.matmul(pbi_p[:], onesr[:], pcar_i[:], start=True, stop=True)
            pbi = sb.tile([128, D], BF16, tag="pbi")
            nc.scalar.copy(pbi[:], pbi_p[:])

            for j in range(NB):
                sl = slice(j * 128, (j + 1) * 128)
                ccp = psm.tile([128, D], F32, tag="med")
                nc.tensor.matmul(ccp[:], qf[:, sl], scar[:], start=True, stop=True)
                t1 = sc.tile([128, D], BF16, tag="t1")
                nc.vector.tensor_mul(t1[:], pbr[:], cosps[j][:])
                t2 = sc.tile([128, D], BF16, tag="t2")
                nc.vector.tensor_mul(t2[:], pbi[:], sinps[j][:])
                s12 = sc.tile([128, D], F32, tag="s12")
                nc.vector.tensor_add(s12[:], t1[:], t2[:])
                a = sc.tile([128, D], F32, tag="a")
                nc.vector.scalar_tensor_tensor(a[:], s12[:], g0p[:, j:j + 1], combs[j][:], AOP.mult, AOP.add)
                comb = sc.tile([128, D], F32, tag="comb")
                nc.vector.scalar_tensor_tensor(comb[:], ccp[:], g1p[:, j:j + 1], a[:], AOP.mult, AOP.add)
                # LayerNorm stats
                zs = sc.tile([128, D], F32, tag="zs")
                ssum = sc.tile([128, 1], F32, tag="ssum")
                nc.scalar.activation(zs[:], comb[:], AFT.Identity, accum_out=ssum[:])
                zq = sc.tile([128, D], F32, tag="zq")
                ssq = sc.tile([128, 1], F32, tag="ssq")
                nc.scalar.activation(zq[:], comb[:], AFT.Square, accum_out=ssq[:])
                mun = sc.tile([128, 1], F32, tag="mun")
                nc.vector.tensor_scalar(mun[:], ssum[:], -1.0 / D, None, AOP.mult)
                mq = sc.tile([128, 1], F32, tag="mq")
                nc.vector.tensor_scalar(mq[:], ssq[:], 1.0 / D, None, AOP.mult)
                mu2 = sc.tile([128, 1], F32, tag="mu2")
                nc.vector.tensor_mul(mu2[:], mun[:], mun[:])
                var = sc.tile([128, 1], F32, tag="var")
                nc.vector.tensor_sub(var[:], mq[:], mu2[:])
                sd = sc.tile([128, 1], F32, tag="sd")
                nc.scalar.activation(sd[:], var[:], AFT.Sqrt, bias=c_eps[:])
                ri = sc.tile([128, 1], F32, tag="ri")
                nc.vector.reciprocal(ri[:], sd[:])
                z = sc.tile([128, D], BF16, tag="z")
                nc.vector.tensor_scalar(z[:], comb[:], mun[:], ri[:], AOP.add, AOP.mult)
                # out = x + z @ Wo' + bo'
                zt = []
                for dt in range(2):
                    tp = psm.tile([128, 128], BF16, tag="medt", bufs=2)
                    nc.tensor.transpose(tp[:], z[:, dt * 128:(dt + 1) * 128], idn[:])
                    zz = sc.tile([128, 128], BF16, tag=f"zt{dt}")
                    nc.vector.tensor_copy(zz[:], tp[:])
                    zt.append(zz)
                op = psm.tile([128, D], F32, tag="med")
                nc.tensor.matmul(op[:], zt[0][:], wo[0][:], start=True, stop=False)
                nc.tensor.matmul(op[:], zt[1][:], wo[1][:], start=False, stop=False)
                nc.tensor.matmul(op[:], onesr[:], bor[:], start=False, stop=True)
                res = sc.tile([128, D], F32, tag="res")
                nc.vector.tensor_add(res[:], op[:], xs[j][:])
                nc.sync.dma_start(out_o[sl, :], res[:])
    nc.compile()
    return nc


_cache = {}


def _get_built():
    if "l1" not in _cache:
        _install_shim()
        _cache["l1"] = _build_l1()
        _cache["l2"] = _build_l2()
    return _cache["l1"], _cache["l2"]


def kernel(**inputs):
    l1, l2 = _get_built()
    inp = {k: np.asarray(v) for k, v in inputs.items()}
    x = inp["x"].astype(np.float32)
    bp = inp["base_phases"].astype(np.float32)

    tri = np.triu(np.ones((128, 128), np.float32))
    onesr = np.ones((1, 128), np.float32)
    idn64 = np.eye(64, dtype=np.float32)
    idn128 = np.eye(128, dtype=np.float32)
    pos_all = np.arange(1, L + 1, dtype=np.float32)

    common1 = dict(
        wk1=inp["Wk1"].astype(BF), wq1=inp["Wq1"].astype(BF),
        wk2=inp["Wk2"].astype(BF), wq2=inp["Wq2"].astype(BF),
        wvc=inp["Wvc"].astype(BF), wvp=inp["Wvp"].astype(BF),
        wg1=inp["Wg1"].astype(BF), wg2=inp["Wg2"].astype(BF),
        bk1=inp["bk1"].reshape(2, 128).T.astype(np.float32),
        bq1=inp["bq1"].reshape(2, 128).T.astype(np.float32),
        bkq2=np.concatenate([inp["bk2"], inp["bq2"]]).reshape(64, 1).astype(np.float32),
        bg1=inp["bg1"].reshape(64, 1).astype(np.float32),
        bvcr=inp["bvc"].reshape(1, D).astype(BF),
        bvpr=inp["bvp"].reshape(1, D).astype(BF),
        bg2r=inp["bg2"].reshape(1, 2).astype(BF),
        trif=tri, trib=tri.astype(BF), onesr=onesr.astype(BF),
        onesc=np.ones((128, 1), np.float32).astype(BF),
        idn64=idn64.astype(BF),
    )
    in1 = []
    for i in range(8):
        b, c = i // 4, i % 4
        rows = slice(c * CH, (c + 1) * CH)
        pos = pos_all[rows]
        m = dict(common1)
        m["xT"] = np.ascontiguousarray(x[b, rows].T).astype(BF)
        m["ph"] = np.ascontiguousarray(bp[rows]).astype(np.float32)
        m["isqp"] = (1.0 / np.sqrt(pos)).reshape(NB, 128).T.astype(np.float32)
        m["isqpk"] = (1.0 / np.sqrt(pos * K)).reshape(NB, 128).T.astype(np.float32)
        in1.append(m)

    r1 = run_bass_kernel_spmd(l1, in1, list(range(8)), trace=PROFILE["trace"])
    if PROFILE["trace"]:
        PROFILE["exec_ns"].append(r1.exec_time_ns)
    res1 = r1.results

    wo_p = (inp["ln_g"][:, None] * inp["Wo"]).astype(BF)
    bo_p = (inp["ln_b"] @ inp["Wo"] + inp["bo"]).reshape(1, D).astype(BF)
    in2 = []
    for i in range(8):
        b, c = i // 4, i % 4
        rows = slice(c * CH, (c + 1) * CH)
        scar = np.zeros((64, D), np.float32)
        pcar = np.zeros((2, D), np.float32)
        for cc in range(c):
            st = res1[b * 4 + cc]["sto"]
            scar += st[0:64]
            pcar += st[64:66]
        m = dict(
            comb=res1[i]["comb"].astype(np.float32),
            qf=res1[i]["qfo"].astype(BF),
            cosp=res1[i]["cospo"].astype(BF),
            sinp=res1[i]["sinpo"].astype(BF),
            scar=scar.astype(BF), pcar=pcar.astype(BF),
            g0=res1[i]["g0o"].astype(np.float32),
            g1=res1[i]["g1o"].astype(np.float32),
            x=np.ascontiguousarray(x[i // 4, slice((i % 4) * CH, (i % 4 + 1) * CH)]),
            wo=wo_p, bor=bo_p, onesr=onesr.astype(BF), idn=idn128.astype(BF),
        )
        in2.append(m)

    r2 = run_bass_kernel_spmd(l2, in2, list(range(8)), trace=PROFILE["trace"])
    if PROFILE["trace"]:
        PROFILE["exec_ns"].append(r2.exec_time_ns)
    res2 = r2.results

    out = np.zeros((B, L, D), np.float32)
    for i in range(8):
        b, c = i // 4, i % 4
        out[b, c * CH:(c + 1) * CH] = res2[i]["out"]
    return out
